# revision 18
# baseline (speedup 1.0000x reference)
"""DynamicsNet Trainium2 kernel: 4 zero-state LSTM cells, data-parallel on 8 cores.

Reference math per row x[16]:
    h1 = relu(lstm1(x));  h2 = selu(lstm2(h1));  m = tanh(lstmM(h2));
    d = tanh(lstmD(h2));  out = concat([m, d], axis=0)
(zero-state LSTM cell: h = sigmoid(o) * tanh(sigmoid(i) * tanh(g)), f unused)

Layout v2: per core, 10 chunk-streams of 12 lanes each -> partitions 0..119
(vs v1's 8 streams x 16 with gap lanes). Each gate bank is a block-diagonal
matmul; cell1 (16 input feats, 10*16 > 128) splits into two 5-chunk halves
whose stationaries write PSUM partitions [0:60) and [60:120) of the same bank.
Biases ride constant-one partitions: rows 80/81 of the x tiles carry literal
1.0 from HBM; lanes 120/121 of the on-chip H tiles are memset to 1.0 on the
(otherwise idle) gpsimd engine. Bias rows are hi/lo fp16 split -> fp32-grade.

Precision: single-term fp16 matmuls (W16 * x16), fp16 activation outputs and
fp16 elementwise; measured end-to-end rel err ~2.5e-3 vs the 2e-2 gate.
Sigmoid goes through tanh (sigma(z) = (1+tanh(z/2))/2 with the 1/2 folded
into weights); selu refactored to (lambda/2 W)*(max(v,0) + 2a*exp(min(v,0)/2)
- 2a) whose -2a centering exactly cancels the selu alpha-offset in the bias.
Outputs written fp16 (tanh range), upcast on host: halves the output DMA.
"""

from contextlib import ExitStack

import numpy as np

LAMBDA = 1.0507009873554805
ALPHA = 1.6732632423543772
TWOA = float(2.0 * ALPHA)

B, IN, H = 1048576, 16, 12
NCORES = 8
R = B // NCORES          # rows per core
NCHUNK = 10              # chunk streams per core
F = 512                  # free-dim tile
NIT = 26                 # iterations; 10*26*512 = 133120 >= 131072 (pad 2048)
CLEN = NIT * F           # 13312 columns per stream
RPAD = NCHUNK * CLEN     # padded rows per core

_CACHED = {}


def _f16(a):
    return np.asarray(a, np.float64).astype(np.float16)


def _prepare_consts(W_ih1, b_ih1, b_hh1, W_ih2, b_ih2, b_hh2,
                    W_ihm, b_ihm, b_hhm, W_ihd, b_ihd, b_hhd):
    i_s, g_s, o_s = slice(0, 12), slice(24, 36), slice(36, 48)
    L2 = LAMBDA / 2.0

    def _c16(v):
        return np.float16(v).astype(np.float64)

    def _chain_v(aI, aG, aO):
        """fp16-simulated H-lane value from gate-bank consts (relu branch)."""
        ti = _c16(np.tanh(aI))
        tg = _c16(np.tanh(aG))
        to = _c16(np.tanh(aO))
        c2 = _c16((ti + 1.0) * tg)
        tc = _c16(np.tanh(0.5 * c2))
        return float(_c16((to + 1.0) * tc))

    A_I, A_G, A_O = 2.0, 0.6505, 2.0
    v1 = _chain_v(A_I, A_G, A_O)          # H1 const-lane value (lanes 60/61)
    # cell2 const seeds ride H1 lanes (value v1); effective bank consts:
    seed = {0: float(_c16(A_I / v1)), 1: float(_c16(A_G / v1)),
            2: float(_c16(A_O / v1))}
    a2_eff = {b: v1 * seed[b] for b in range(3)}
    vh2 = _chain_v(a2_eff[0], a2_eff[1], a2_eff[2])
    e2c = _c16(2.0 * ALPHA)
    v2 = float(_c16(vh2 + float(e2c) - 2.0 * ALPHA))  # H2 lanes 120/121

    b1 = (b_ih1 + b_hh1).astype(np.float64)
    b2 = (b_ih2 + b_hh2).astype(np.float64)
    bm = (b_ihm + b_hhm).astype(np.float64)
    bd = (b_ihd + b_hhd).astype(np.float64)

    # slot layout in w_dram: [c1A x3 banks, c1B x3, cell2 x3, m x3, d x3]
    w_np = np.zeros((128, 15 * 128), np.float16)

    def put_bias(slot, cols12, bb, scale_lane, r0=120):
        bbv = np.asarray(bb, np.float64) / scale_lane
        hi = _f16(bbv)
        lo = _f16(bbv - hi.astype(np.float64))
        for j0 in cols12:
            w_np[r0, 128 * slot + j0:128 * slot + j0 + 12] = hi
            w_np[r0 + 1, 128 * slot + j0:128 * slot + j0 + 12] = lo

    for b, (gsl, sc) in enumerate(((i_s, 0.5), (g_s, 1.0), (o_s, 0.5))):
        W1b = _f16(W_ih1.astype(np.float64)[gsl] * sc)          # [12, 16]
        bhi = _f16(b1[gsl] * sc)
        blo = _f16(b1[gsl] * sc - bhi.astype(np.float64))
        for half, slot in ((0, b), (1, 3 + b)):
            for j in range(5):
                c0 = 128 * slot + 12 * j
                w_np[16 * j:16 * j + 16, c0:c0 + 12] = W1b.T
                w_np[80, c0:c0 + 12] = bhi
                w_np[81, c0:c0 + 12] = blo
        # A-half const cols 60/61 -> bank lanes 60/61 (seed the H1 const chain)
        w_np[80, 128 * b + 60:128 * b + 62] = np.float16((A_I, A_G, A_O)[b])
        # cell2: input H1 is holey (chunks 0-4 at rows 0..59, hole 60..63
        # with memset-1.0 bias lanes at 60/61, chunks 5-9 at rows 64..123)
        slot = 6 + b
        Wb = _f16(W_ih2.astype(np.float64)[gsl] * sc * 0.5)     # [12, 12]
        b2hi = _f16(b2[gsl] * sc / v1)
        b2lo = _f16(b2[gsl] * sc / v1 - b2hi.astype(np.float64))
        for j in range(NCHUNK):
            r0 = 12 * j if j < 5 else 64 + 12 * (j - 5)
            c0 = 128 * slot + 12 * j
            w_np[r0:r0 + 12, c0:c0 + 12] = Wb.T
            w_np[60, c0:c0 + 12] = b2hi
            w_np[61, c0:c0 + 12] = b2lo
        # cell2 const cols 120/121 (ride H1 lane value v1 -> bank consts)
        w_np[60, 128 * slot + 120:128 * slot + 122] = np.float16(seed[b])
        # m/d: input H2 dense, bias on const lanes 120/121
        for cell, (W, bb) in enumerate(((W_ihm, bm), (W_ihd, bd))):
            slot = 9 + 3 * cell + b
            Wb = _f16(W.astype(np.float64)[gsl] * sc * L2)      # [12, 12]
            for j in range(NCHUNK):
                c0 = 128 * slot + 12 * j
                w_np[12 * j:12 * j + 12, c0:c0 + 12] = Wb.T
        for cell, bb in enumerate((bm, bd)):
            put_bias(9 + 3 * cell + b, range(0, 120, 12), bb[gsl] * sc, v2)
    return (w_np,)


def _build_bass():
    import concourse.bass as bass
    import concourse.mybir as mybir
    import concourse.tile as tile

    fp32 = mybir.dt.float32
    fp16 = mybir.dt.float16
    Tanh = mybir.ActivationFunctionType.Tanh
    Exp = mybir.ActivationFunctionType.Exp
    ADD = mybir.AluOpType.add
    MULT = mybir.AluOpType.mult
    MAX = mybir.AluOpType.max
    MIN = mybir.AluOpType.min

    P = 120              # dense data partitions (cell2/m/d outputs)
    PA = 124             # holey cell1 span: [0:60) + hole + [64:124)

    nc = bass.Bass(name="dynet2")
    x_dev = nc.dram_tensor("x_dev", [82, 2 * CLEN], fp16,
                           kind="ExternalInput")
    w_dram = nc.dram_tensor("w_dram", [128, 15 * 128], fp16,
                            kind="ExternalInput")
    md_dev = nc.dram_tensor("md_dev", [P, 2 * CLEN], fp16,
                            kind="ExternalOutput")

    with tile.TileContext(nc) as tc, ExitStack() as ctx:
        const_p = ctx.enter_context(tc.tile_pool(name="const", bufs=1))
        xp = ctx.enter_context(tc.tile_pool(name="x", bufs=3))
        Tp = ctx.enter_context(tc.tile_pool(name="T", bufs=3))
        smallp = ctx.enter_context(tc.tile_pool(name="small", bufs=4))
        hp = ctx.enter_context(tc.tile_pool(name="h", bufs=4))
        psp = ctx.enter_context(tc.tile_pool(name="ps", bufs=1, space="PSUM"))

        wsb = const_p.tile([128, 15 * 128], fp16)
        for q in range(4):
            c0, c1 = 480 * q, 480 * (q + 1)
            nc.sync.dma_start(wsb[:, c0:c1], w_dram[:, c0:c1])
        ebt = const_p.tile([128, 1], fp32)
        nc.vector.memset(ebt[:], float(np.log(2.0 * ALPHA)))

        # PSUM ring of 8 banks; each stage claims 3 consecutive (mod 8).
        # cell1's A-half spans 64 cols (62/63 are zero weight columns), so
        # every lane a matmul ever reads back is written finite first.
        pp_all = psp.tile([128, 4096], fp32)

        def bank(s, b):
            w = ((3 * s + b) % 8) * 512
            return pp_all[:, w:w + 512]

        def mms_h(s, slot0, rhs, nrow, ncol=120):
            """3 block-diag matmuls (one per bank) for an h-input cell."""
            for b in range(3):
                w = 128 * (slot0 + b)
                nc.tensor.matmul(
                    bank(s, b)[0:ncol, :], wsb[0:nrow, w:w + ncol],
                    rhs[0:nrow, :], start=True, stop=True)

        def mms_x(s, xt):
            """cell1: two 5-chunk halves per bank -> partitions [0:62),[64:124)."""
            for b in range(3):
                wa, wb = 128 * b, 128 * (3 + b)
                nc.tensor.matmul(
                    bank(s, b)[0:64, :], wsb[0:82, wa:wa + 64],
                    xt[0:82, 0:F], start=True, stop=True)
                nc.tensor.matmul(
                    bank(s, b)[64:124, :], wsb[0:82, wb:wb + 60],
                    xt[0:82, F:2 * F], start=True, stop=True)

        def ring_act(T, tcol0, base, nq, pm=P):
            """tanh over ring slots [base, base+nq) -> T[:, tcol0:+512*nq]."""
            done = 0
            while done < nq:
                b0 = (base + done) % 8
                n1 = min(nq - done, 8 - b0)
                nc.scalar.activation(
                    T[0:pm, tcol0 + 512 * done:tcol0 + 512 * (done + n1)],
                    pp_all[0:pm, 512 * b0:512 * (b0 + n1)], Tanh)
                done += n1

        def gate_act(s, T, pm=P):
            ring_act(T, 0, (3 * s) % 8, 3, pm=pm)

        xa = {}
        H1 = {}
        H2 = {}
        sctr = 0
        for k in range(NIT + 3):
            if k < NIT:
                xa[k] = xp.tile([82, 2 * F], fp16, tag="xa", name=f"xa{k}")
                nc.sync.dma_start(xa[k][0:41, :],
                                  x_dev[0:41, 2 * F * k:2 * F * (k + 1)])
                nc.sync.dma_start(xa[k][41:82, :],
                                  x_dev[41:82, 2 * F * k:2 * F * (k + 1)])

            # fused tanh in/out: [0:1024]=hmd(k-2), [1024:2048]=c2both(k-1,k)
            fin = smallp.tile([PA, 2048], fp16, tag="fin", name=f"fin{k}")
            fout = smallp.tile([PA, 2048], fp16, tag="fout", name=f"fout{k}")
            hmd = fin[:, 0:1024]
            omd = fout[:, 0:1024]

            # --- stage m/d for iteration k-3 ---
            if 3 <= k:
                it = k - 3
                sm, sd = sctr, sctr + 1
                sctr += 2
                mms_h(sm, 9, H2[it][:], 122)
                mms_h(sd, 12, H2[it][:], 122)
                Tmd = Tp.tile([P, 3072], fp16, tag="Tmd", bufs=2)
                Tm = Tmd[:, 0:1536]
                Td = Tmd[:, 1536:3072]
                c2md = smallp.tile([P, 1024], fp16, tag="c2md")
                tcmd = smallp.tile([P, 1024], fp16, tag="tcmd")
                if (3 * sm) % 8 <= 2:
                    ring_act(Tmd, 0, (3 * sm) % 8, 6)
                else:
                    gate_act(sm, Tm)
                    gate_act(sd, Td)
                nc.vector.scalar_tensor_tensor(
                    c2md[:, 0:512], Tm[:, 0:512], 1.0, Tm[:, 512:1024],
                    op0=ADD, op1=MULT)
                nc.vector.scalar_tensor_tensor(
                    c2md[:, 512:1024], Td[:, 0:512], 1.0, Td[:, 512:1024],
                    op0=ADD, op1=MULT)
                nc.scalar.activation(tcmd[:, :], c2md[:, :], Tanh, scale=0.5)
                nc.vector.scalar_tensor_tensor(
                    fin[0:P, 0:512], Tm[:, 1024:1536], 1.0, tcmd[:, 0:512],
                    op0=ADD, op1=MULT)
                nc.vector.scalar_tensor_tensor(
                    fin[0:P, 512:1024], Td[:, 1024:1536], 1.0,
                    tcmd[:, 512:1024], op0=ADD, op1=MULT)
                del H2[it]
                md_out_it = it

            # --- stage cell2 for iteration k-1 ---
            if 1 <= k <= NIT:
                it = k - 1
                s2 = sctr
                sctr += 1
                mms_h(s2, 6, H1[it][:], 124, ncol=122)
                T2 = Tp.tile([PA, 1536], fp16, tag="T")
                gate_act(s2, T2, pm=122)
                nc.vector.scalar_tensor_tensor(
                    fin[0:122, 1024:1536], T2[0:122, 0:512], 1.0,
                    T2[0:122, 512:1024], op0=ADD, op1=MULT)

            # --- stage cell1 for iteration k ---
            if k < NIT:
                s1 = sctr
                sctr += 1
                mms_x(s1, xa[k])
                T1 = Tp.tile([PA, 1536], fp16, tag="T")
                gate_act(s1, T1, pm=PA)
                nc.vector.scalar_tensor_tensor(
                    fin[0:PA, 1536:2048], T1[0:PA, 0:512], 1.0,
                    T1[0:PA, 512:1024], op0=ADD, op1=MULT)

            # merged tanh(0.5*x) over [hmd(k-3) | c2_2(k-1) | c2_1(k)]
            lo = 0 if 3 <= k else 1024
            hi = 2048 if k < NIT else (1536 if k <= NIT else 1024)
            if k == 0:
                lo = 1536
            nc.scalar.activation(fout[0:PA, lo:hi], fin[0:PA, lo:hi], Tanh,
                                 scale=0.5)
            if 3 <= k:
                it = md_out_it
                nc.gpsimd.dma_start(md_dev[0:60, 2 * F * it:2 * F * (it + 1)],
                                    omd[0:60, 0:1024])
                nc.gpsimd.dma_start(md_dev[60:120, 2 * F * it:2 * F * (it + 1)],
                                    omd[60:120, 0:1024])

            # --- cell2 epilogue: selu -> H2 fp16 (+const lanes) ---
            if 1 <= k <= NIT:
                it = k - 1
                h2x2 = smallp.tile([122, F], fp16, tag="h2x2",
                                   name=f"h2x2_{k}")
                nc.vector.scalar_tensor_tensor(
                    h2x2[:, :], T2[0:122, 1024:1536], 1.0,
                    fout[0:122, 1024:1536], op0=ADD, op1=MULT)
                m0t = smallp.tile([122, F], fp16, tag="m0")
                e2t = smallp.tile([122, F], fp16, tag="e2")
                tmp2 = smallp.tile([122, F], fp32, tag="tmp2")
                h2t = hp.tile([128, F], fp16, tag="H2", name=f"H2_{it}")
                nc.vector.tensor_scalar_min(m0t[:, :], h2x2[:, :], 0.0)
                nc.scalar.activation(e2t[:, :], m0t[:, :], Exp,
                                     bias=ebt[0:122, :], scale=0.5)
                nc.vector.scalar_tensor_tensor(
                    tmp2[:, :], h2x2[:, :], 0.0, e2t[:, :], op0=MAX, op1=ADD)
                nc.vector.tensor_scalar_sub(h2t[0:122, :], tmp2[:, :], TWOA)
                H2[it] = h2t
                del H1[it]

            # --- cell1 epilogue: relu -> H1 fp16 (+const lanes) ---
            if k < NIT:
                h2x1 = smallp.tile([PA, F], fp16, tag="h2x1",
                                   name=f"h2x1_{k}")
                nc.vector.scalar_tensor_tensor(
                    h2x1[:, :], T1[0:PA, 1024:1536], 1.0,
                    fout[0:PA, 1536:2048], op0=ADD, op1=MULT)
                h1t = hp.tile([128, F], fp16, tag="H1", name=f"H1_{k}")
                nc.vector.tensor_scalar_max(h1t[0:PA, :], h2x1[:, :], 0.0)
                H1[k] = h1t
                del xa[k]

    _legalize_waits(nc)
    return nc


def _legalize_waits(nc):
    """Split multi-wait instructions into single-wait same-engine NoOps
    (the cayman ISA has one sync-wait slot per instruction)."""
    import concourse.mybir as mybir
    n = 0
    for func in nc.m.functions:
        for blk in func.blocks:
            out = []
            changed = False
            for inst in blk.instructions:
                si = inst.sync_info
                waits = list(si.on_wait) if si is not None and si.on_wait else []
                if len(waits) > 1:
                    changed = True
                    for w in waits[:-1]:
                        n += 1
                        nop = mybir.InstNoOp(name=f"legw-{n}", ins=[], outs=[])
                        nop.engine = inst.engine
                        nop.sync_info = mybir.SyncInfo(on_wait=[w], on_update=[])
                        out.append(nop)
                    inst.sync_info = mybir.SyncInfo(
                        on_wait=[waits[-1]],
                        on_update=list(si.on_update) if si.on_update else [])
                out.append(inst)
            if changed:
                blk.instructions = out
    return n


def _run(x, consts, trace=False):
    from concourse.bass_utils import run_bass_kernel_spmd

    if "nc" not in _CACHED:
        _CACHED["nc"] = _build_bass()
    nc = _CACHED["nc"]
    (w_np,) = consts

    in_maps = []
    for c in range(NCORES):
        xpad = np.zeros((RPAD, IN), np.float32)
        xpad[:R] = x[c * R:(c + 1) * R]
        blk = np.ascontiguousarray(
            xpad.reshape(NCHUNK, CLEN, IN).transpose(0, 2, 1))  # [10,16,CLEN]
        xa = np.empty((82, CLEN), np.float16)
        xb = np.empty((82, CLEN), np.float16)
        for j in range(5):
            xa[16 * j:16 * j + 16] = blk[j]
            xb[16 * j:16 * j + 16] = blk[5 + j]
        xa[80:82] = 1.0
        xb[80:82] = 1.0
        xdv = np.empty((82, 2 * CLEN), np.float16)
        xv = xdv.reshape(82, NIT, 2 * F)
        xv[:, :, 0:F] = xa.reshape(82, NIT, F)
        xv[:, :, F:2 * F] = xb.reshape(82, NIT, F)
        in_maps.append({"x_dev": xdv, "w_dram": w_np})

    res = run_bass_kernel_spmd(nc, in_maps, core_ids=list(range(NCORES)),
                               trace=trace)

    out = np.empty((2 * B, H), np.float32)
    for c in range(NCORES):
        md = res.results[c]["md_dev"].reshape(120, NIT, 2 * F)
        for half, base in ((0, 0), (1, B)):
            dev = np.ascontiguousarray(
                md[:, :, half * F:(half + 1) * F]).reshape(120, CLEN)
            full = dev.reshape(NCHUNK, H, CLEN).transpose(0, 2, 1)
            out[base + c * R: base + (c + 1) * R] = (
                full.reshape(RPAD, H)[:R].astype(np.float32))
    return out, res


def kernel(x, W_ih1, b_ih1, b_hh1, W_ih2, b_ih2, b_hh2,
           W_ihm, b_ihm, b_hhm, W_ihd, b_ihd, b_hhd):
    x = np.asarray(x, np.float32)
    consts = _prepare_consts(
        np.asarray(W_ih1, np.float32), np.asarray(b_ih1, np.float32),
        np.asarray(b_hh1, np.float32), np.asarray(W_ih2, np.float32),
        np.asarray(b_ih2, np.float32), np.asarray(b_hh2, np.float32),
        np.asarray(W_ihm, np.float32), np.asarray(b_ihm, np.float32),
        np.asarray(b_hhm, np.float32), np.asarray(W_ihd, np.float32),
        np.asarray(b_ihd, np.float32), np.asarray(b_hhd, np.float32))
    out, _ = _run(x, consts, trace=False)
    return out


# revision 19
# speedup vs baseline: 1.0283x; 1.0283x over previous
"""DynamicsNet Trainium2 kernel: 4 zero-state LSTM cells, data-parallel on 8 cores.

Reference math per row x[16]:
    h1 = relu(lstm1(x));  h2 = selu(lstm2(h1));  m = tanh(lstmM(h2));
    d = tanh(lstmD(h2));  out = concat([m, d], axis=0)
(zero-state LSTM cell: h = sigmoid(o) * tanh(sigmoid(i) * tanh(g)), f unused)

Layout v2: per core, 10 chunk-streams of 12 lanes each -> partitions 0..119
(vs v1's 8 streams x 16 with gap lanes). Each gate bank is a block-diagonal
matmul; cell1 (16 input feats, 10*16 > 128) splits into two 5-chunk halves
whose stationaries write PSUM partitions [0:60) and [60:120) of the same bank.
Biases ride constant-one partitions: rows 80/81 of the x tiles carry literal
1.0 from HBM; lanes 120/121 of the on-chip H tiles are memset to 1.0 on the
(otherwise idle) gpsimd engine. Bias rows are hi/lo fp16 split -> fp32-grade.

Precision: single-term fp16 matmuls (W16 * x16), fp16 activation outputs and
fp16 elementwise; measured end-to-end rel err ~2.5e-3 vs the 2e-2 gate.
Sigmoid goes through tanh (sigma(z) = (1+tanh(z/2))/2 with the 1/2 folded
into weights); selu refactored to (lambda/2 W)*(max(v,0) + 2a*exp(min(v,0)/2)
- 2a) whose -2a centering exactly cancels the selu alpha-offset in the bias.
Outputs written fp16 (tanh range), upcast on host: halves the output DMA.
"""

from contextlib import ExitStack

import numpy as np

LAMBDA = 1.0507009873554805
ALPHA = 1.6732632423543772
TWOA = float(2.0 * ALPHA)

B, IN, H = 1048576, 16, 12
NCORES = 8
R = B // NCORES          # rows per core
NCHUNK = 10              # chunk streams per core
F = 512                  # free-dim tile
NIT = 26                 # iterations; 10*26*512 = 133120 >= 131072 (pad 2048)
CLEN = NIT * F           # 13312 columns per stream
RPAD = NCHUNK * CLEN     # padded rows per core

_CACHED = {}


def _f16(a):
    return np.asarray(a, np.float64).astype(np.float16)


def _prepare_consts(W_ih1, b_ih1, b_hh1, W_ih2, b_ih2, b_hh2,
                    W_ihm, b_ihm, b_hhm, W_ihd, b_ihd, b_hhd):
    i_s, g_s, o_s = slice(0, 12), slice(24, 36), slice(36, 48)
    L2 = LAMBDA / 2.0

    def _c16(v):
        return np.float16(v).astype(np.float64)

    def _chain_v(aI, aG, aO):
        """fp16-simulated H-lane value from gate-bank consts (relu branch)."""
        ti = _c16(np.tanh(aI))
        tg = _c16(np.tanh(aG))
        to = _c16(np.tanh(aO))
        c2 = _c16((ti + 1.0) * tg)
        tc = _c16(np.tanh(0.5 * c2))
        return float(_c16((to + 1.0) * tc))

    A_I, A_G, A_O = 2.0, 0.6505, 2.0
    v1 = _chain_v(A_I, A_G, A_O)          # H1 const-lane value (lanes 60/61)
    # cell2 const seeds ride H1 lanes (value v1); effective bank consts:
    seed = {0: float(_c16(A_I / v1)), 1: float(_c16(A_G / v1)),
            2: float(_c16(A_O / v1))}
    a2_eff = {b: v1 * seed[b] for b in range(3)}
    vh2 = _chain_v(a2_eff[0], a2_eff[1], a2_eff[2])
    e2c = _c16(2.0 * ALPHA)
    v2 = float(_c16(vh2 + float(e2c) - 2.0 * ALPHA))  # H2 lanes 120/121

    b1 = (b_ih1 + b_hh1).astype(np.float64)
    b2 = (b_ih2 + b_hh2).astype(np.float64)
    bm = (b_ihm + b_hhm).astype(np.float64)
    bd = (b_ihd + b_hhd).astype(np.float64)

    # slot layout in w_dram: [c1A x3 banks, c1B x3, cell2 x3, m x3, d x3]
    w_np = np.zeros((128, 15 * 128), np.float16)

    def put_bias(slot, cols12, bb, scale_lane, r0=120):
        bbv = np.asarray(bb, np.float64) / scale_lane
        hi = _f16(bbv)
        lo = _f16(bbv - hi.astype(np.float64))
        for j0 in cols12:
            w_np[r0, 128 * slot + j0:128 * slot + j0 + 12] = hi
            w_np[r0 + 1, 128 * slot + j0:128 * slot + j0 + 12] = lo

    for b, (gsl, sc) in enumerate(((i_s, 0.5), (g_s, 1.0), (o_s, 0.5))):
        W1b = _f16(W_ih1.astype(np.float64)[gsl] * sc)          # [12, 16]
        bhi = _f16(b1[gsl] * sc)
        blo = _f16(b1[gsl] * sc - bhi.astype(np.float64))
        for half, slot in ((0, b), (1, 3 + b)):
            for j in range(5):
                c0 = 128 * slot + 12 * j
                w_np[16 * j:16 * j + 16, c0:c0 + 12] = W1b.T
                w_np[80, c0:c0 + 12] = bhi
                w_np[81, c0:c0 + 12] = blo
        # A-half const cols 60/61 -> bank lanes 60/61 (seed the H1 const chain)
        w_np[80, 128 * b + 60:128 * b + 62] = np.float16((A_I, A_G, A_O)[b])
        # cell2: input H1 is holey (chunks 0-4 at rows 0..59, hole 60..63
        # with memset-1.0 bias lanes at 60/61, chunks 5-9 at rows 64..123)
        slot = 6 + b
        Wb = _f16(W_ih2.astype(np.float64)[gsl] * sc * 0.5)     # [12, 12]
        b2hi = _f16(b2[gsl] * sc / v1)
        b2lo = _f16(b2[gsl] * sc / v1 - b2hi.astype(np.float64))
        for j in range(NCHUNK):
            r0 = 12 * j if j < 5 else 64 + 12 * (j - 5)
            c0 = 128 * slot + 12 * j
            w_np[r0:r0 + 12, c0:c0 + 12] = Wb.T
            w_np[60, c0:c0 + 12] = b2hi
            w_np[61, c0:c0 + 12] = b2lo
        # cell2 const cols 120/121 (ride H1 lane value v1 -> bank consts)
        w_np[60, 128 * slot + 120:128 * slot + 122] = np.float16(seed[b])
        # m/d: input H2 dense, bias on const lanes 120/121
        for cell, (W, bb) in enumerate(((W_ihm, bm), (W_ihd, bd))):
            slot = 9 + 3 * cell + b
            Wb = _f16(W.astype(np.float64)[gsl] * sc * L2)      # [12, 12]
            for j in range(NCHUNK):
                c0 = 128 * slot + 12 * j
                w_np[12 * j:12 * j + 12, c0:c0 + 12] = Wb.T
        for cell, bb in enumerate((bm, bd)):
            put_bias(9 + 3 * cell + b, range(0, 120, 12), bb[gsl] * sc, v2)
    return (w_np,)


def _build_bass():
    import concourse.bass as bass
    import concourse.mybir as mybir
    import concourse.tile as tile

    fp32 = mybir.dt.float32
    fp16 = mybir.dt.float16
    Tanh = mybir.ActivationFunctionType.Tanh
    Exp = mybir.ActivationFunctionType.Exp
    ADD = mybir.AluOpType.add
    MULT = mybir.AluOpType.mult
    MAX = mybir.AluOpType.max
    MIN = mybir.AluOpType.min

    P = 120              # dense data partitions (cell2/m/d outputs)
    PA = 124             # holey cell1 span: [0:60) + hole + [64:124)

    nc = bass.Bass(name="dynet2")
    x_dev = nc.dram_tensor("x_dev", [82, 2 * CLEN], fp16,
                           kind="ExternalInput")
    w_dram = nc.dram_tensor("w_dram", [128, 15 * 128], fp16,
                            kind="ExternalInput")
    md_dev = nc.dram_tensor("md_dev", [P, 2 * CLEN], fp16,
                            kind="ExternalOutput")

    with tile.TileContext(nc) as tc, ExitStack() as ctx:
        const_p = ctx.enter_context(tc.tile_pool(name="const", bufs=1))
        xp = ctx.enter_context(tc.tile_pool(name="x", bufs=3))
        Tp = ctx.enter_context(tc.tile_pool(name="T", bufs=3))
        smallp = ctx.enter_context(tc.tile_pool(name="small", bufs=4))
        hp = ctx.enter_context(tc.tile_pool(name="h", bufs=4))
        psp = ctx.enter_context(tc.tile_pool(name="ps", bufs=1, space="PSUM"))

        wsb = const_p.tile([128, 15 * 128], fp16)
        for q in range(4):
            c0, c1 = 480 * q, 480 * (q + 1)
            nc.sync.dma_start(wsb[:, c0:c1], w_dram[:, c0:c1])
        ebt = const_p.tile([128, 1], fp32)
        nc.vector.memset(ebt[:], float(np.log(2.0 * ALPHA)))

        # PSUM ring of 8 banks; each stage claims 3 consecutive (mod 8).
        # cell1's A-half spans 64 cols (62/63 are zero weight columns), so
        # every lane a matmul ever reads back is written finite first.
        pp_all = psp.tile([128, 4096], fp32)

        def bank(s, b):
            w = ((3 * s + b) % 8) * 512
            return pp_all[:, w:w + 512]

        def mms_h(s, slot0, rhs, nrow, ncol=120):
            """3 block-diag matmuls (one per bank) for an h-input cell."""
            for b in range(3):
                w = 128 * (slot0 + b)
                nc.tensor.matmul(
                    bank(s, b)[0:ncol, :], wsb[0:nrow, w:w + ncol],
                    rhs[0:nrow, :], start=True, stop=True)

        def mms_x(s, xt):
            """cell1: two 5-chunk halves per bank -> partitions [0:62),[64:124)."""
            for b in range(3):
                wa, wb = 128 * b, 128 * (3 + b)
                nc.tensor.matmul(
                    bank(s, b)[0:64, :], wsb[0:82, wa:wa + 64],
                    xt[0:82, 0:F], start=True, stop=True)
                nc.tensor.matmul(
                    bank(s, b)[64:124, :], wsb[0:82, wb:wb + 60],
                    xt[0:82, F:2 * F], start=True, stop=True)

        def ring_act(T, tcol0, base, nq, pm=P):
            """tanh over ring slots [base, base+nq) -> T[:, tcol0:+512*nq]."""
            done = 0
            while done < nq:
                b0 = (base + done) % 8
                n1 = min(nq - done, 8 - b0)
                nc.scalar.activation(
                    T[0:pm, tcol0 + 512 * done:tcol0 + 512 * (done + n1)],
                    pp_all[0:pm, 512 * b0:512 * (b0 + n1)], Tanh)
                done += n1

        def gate_act(s, T, pm=P):
            ring_act(T, 0, (3 * s) % 8, 3, pm=pm)

        xa = {}
        H1 = {}
        H2 = {}
        sctr = 0
        for k in range(NIT + 3):
            if k < NIT:
                xa[k] = xp.tile([82, 2 * F], fp16, tag="xa", name=f"xa{k}")
                nc.sync.dma_start(xa[k][:],
                                  x_dev[:, 2 * F * k:2 * F * (k + 1)])

            # fused tanh in/out: [0:1024]=hmd(k-2), [1024:2048]=c2both(k-1,k)
            fin = smallp.tile([PA, 2048], fp16, tag="fin", name=f"fin{k}")
            fout = smallp.tile([PA, 2048], fp16, tag="fout", name=f"fout{k}")
            hmd = fin[:, 0:1024]
            omd = fout[:, 0:1024]

            # --- stage m/d for iteration k-3 ---
            if 3 <= k:
                it = k - 3
                sm, sd = sctr, sctr + 1
                sctr += 2
                mms_h(sm, 9, H2[it][:], 122)
                mms_h(sd, 12, H2[it][:], 122)
                Tmd = Tp.tile([P, 3072], fp16, tag="Tmd", bufs=2)
                Tm = Tmd[:, 0:1536]
                Td = Tmd[:, 1536:3072]
                c2md = smallp.tile([P, 1024], fp16, tag="c2md")
                tcmd = smallp.tile([P, 1024], fp16, tag="tcmd")
                if (3 * sm) % 8 <= 2:
                    ring_act(Tmd, 0, (3 * sm) % 8, 6)
                else:
                    gate_act(sm, Tm)
                    gate_act(sd, Td)
                nc.vector.scalar_tensor_tensor(
                    c2md[:, 0:512], Tm[:, 0:512], 1.0, Tm[:, 512:1024],
                    op0=ADD, op1=MULT)
                nc.vector.scalar_tensor_tensor(
                    c2md[:, 512:1024], Td[:, 0:512], 1.0, Td[:, 512:1024],
                    op0=ADD, op1=MULT)
                nc.scalar.activation(tcmd[:, :], c2md[:, :], Tanh, scale=0.5)
                nc.vector.scalar_tensor_tensor(
                    fin[0:P, 0:512], Tm[:, 1024:1536], 1.0, tcmd[:, 0:512],
                    op0=ADD, op1=MULT)
                nc.vector.scalar_tensor_tensor(
                    fin[0:P, 512:1024], Td[:, 1024:1536], 1.0,
                    tcmd[:, 512:1024], op0=ADD, op1=MULT)
                del H2[it]
                md_out_it = it

            # --- stage cell2 for iteration k-1 ---
            if 1 <= k <= NIT:
                it = k - 1
                s2 = sctr
                sctr += 1
                mms_h(s2, 6, H1[it][:], 124, ncol=122)
                T2 = Tp.tile([PA, 1536], fp16, tag="T")
                gate_act(s2, T2, pm=122)
                nc.vector.scalar_tensor_tensor(
                    fin[0:122, 1024:1536], T2[0:122, 0:512], 1.0,
                    T2[0:122, 512:1024], op0=ADD, op1=MULT)

            # --- stage cell1 for iteration k ---
            if k < NIT:
                s1 = sctr
                sctr += 1
                mms_x(s1, xa[k])
                T1 = Tp.tile([PA, 1536], fp16, tag="T")
                gate_act(s1, T1, pm=PA)
                nc.vector.scalar_tensor_tensor(
                    fin[0:PA, 1536:2048], T1[0:PA, 0:512], 1.0,
                    T1[0:PA, 512:1024], op0=ADD, op1=MULT)

            # merged tanh(0.5*x) over [hmd(k-3) | c2_2(k-1) | c2_1(k)]
            lo = 0 if 3 <= k else 1024
            hi = 2048 if k < NIT else (1536 if k <= NIT else 1024)
            if k == 0:
                lo = 1536
            nc.scalar.activation(fout[0:PA, lo:hi], fin[0:PA, lo:hi], Tanh,
                                 scale=0.5)
            if 3 <= k:
                it = md_out_it
                nc.gpsimd.dma_start(md_dev[:, 2 * F * it:2 * F * (it + 1)],
                                    omd[0:P, 0:1024])

            # --- cell2 epilogue: selu -> H2 fp16 (+const lanes) ---
            if 1 <= k <= NIT:
                it = k - 1
                h2x2 = smallp.tile([122, F], fp16, tag="h2x2",
                                   name=f"h2x2_{k}")
                nc.vector.scalar_tensor_tensor(
                    h2x2[:, :], T2[0:122, 1024:1536], 1.0,
                    fout[0:122, 1024:1536], op0=ADD, op1=MULT)
                m0t = smallp.tile([122, F], fp16, tag="m0")
                e2t = smallp.tile([122, F], fp16, tag="e2")
                tmp2 = smallp.tile([122, F], fp32, tag="tmp2")
                h2t = hp.tile([128, F], fp16, tag="H2", name=f"H2_{it}")
                nc.vector.tensor_scalar_min(m0t[:, :], h2x2[:, :], 0.0)
                nc.scalar.activation(e2t[:, :], m0t[:, :], Exp,
                                     bias=ebt[0:122, :], scale=0.5)
                nc.vector.scalar_tensor_tensor(
                    tmp2[:, :], h2x2[:, :], 0.0, e2t[:, :], op0=MAX, op1=ADD)
                nc.vector.tensor_scalar_sub(h2t[0:122, :], tmp2[:, :], TWOA)
                H2[it] = h2t
                del H1[it]

            # --- cell1 epilogue: relu -> H1 fp16 (+const lanes) ---
            if k < NIT:
                h2x1 = smallp.tile([PA, F], fp16, tag="h2x1",
                                   name=f"h2x1_{k}")
                nc.vector.scalar_tensor_tensor(
                    h2x1[:, :], T1[0:PA, 1024:1536], 1.0,
                    fout[0:PA, 1536:2048], op0=ADD, op1=MULT)
                h1t = hp.tile([128, F], fp16, tag="H1", name=f"H1_{k}")
                nc.vector.tensor_scalar_max(h1t[0:PA, :], h2x1[:, :], 0.0)
                H1[k] = h1t
                del xa[k]

    _legalize_waits(nc)
    return nc


def _legalize_waits(nc):
    """Split multi-wait instructions into single-wait same-engine NoOps
    (the cayman ISA has one sync-wait slot per instruction)."""
    import concourse.mybir as mybir
    n = 0
    for func in nc.m.functions:
        for blk in func.blocks:
            out = []
            changed = False
            for inst in blk.instructions:
                si = inst.sync_info
                waits = list(si.on_wait) if si is not None and si.on_wait else []
                if len(waits) > 1:
                    changed = True
                    for w in waits[:-1]:
                        n += 1
                        nop = mybir.InstNoOp(name=f"legw-{n}", ins=[], outs=[])
                        nop.engine = inst.engine
                        nop.sync_info = mybir.SyncInfo(on_wait=[w], on_update=[])
                        out.append(nop)
                    inst.sync_info = mybir.SyncInfo(
                        on_wait=[waits[-1]],
                        on_update=list(si.on_update) if si.on_update else [])
                out.append(inst)
            if changed:
                blk.instructions = out
    return n


def _run(x, consts, trace=False):
    from concourse.bass_utils import run_bass_kernel_spmd

    if "nc" not in _CACHED:
        _CACHED["nc"] = _build_bass()
    nc = _CACHED["nc"]
    (w_np,) = consts

    in_maps = []
    for c in range(NCORES):
        xpad = np.zeros((RPAD, IN), np.float32)
        xpad[:R] = x[c * R:(c + 1) * R]
        blk = np.ascontiguousarray(
            xpad.reshape(NCHUNK, CLEN, IN).transpose(0, 2, 1))  # [10,16,CLEN]
        xa = np.empty((82, CLEN), np.float16)
        xb = np.empty((82, CLEN), np.float16)
        for j in range(5):
            xa[16 * j:16 * j + 16] = blk[j]
            xb[16 * j:16 * j + 16] = blk[5 + j]
        xa[80:82] = 1.0
        xb[80:82] = 1.0
        xdv = np.empty((82, 2 * CLEN), np.float16)
        xv = xdv.reshape(82, NIT, 2 * F)
        xv[:, :, 0:F] = xa.reshape(82, NIT, F)
        xv[:, :, F:2 * F] = xb.reshape(82, NIT, F)
        in_maps.append({"x_dev": xdv, "w_dram": w_np})

    res = run_bass_kernel_spmd(nc, in_maps, core_ids=list(range(NCORES)),
                               trace=trace)

    out = np.empty((2 * B, H), np.float32)
    for c in range(NCORES):
        md = res.results[c]["md_dev"].reshape(120, NIT, 2 * F)
        for half, base in ((0, 0), (1, B)):
            dev = np.ascontiguousarray(
                md[:, :, half * F:(half + 1) * F]).reshape(120, CLEN)
            full = dev.reshape(NCHUNK, H, CLEN).transpose(0, 2, 1)
            out[base + c * R: base + (c + 1) * R] = (
                full.reshape(RPAD, H)[:R].astype(np.float32))
    return out, res


def kernel(x, W_ih1, b_ih1, b_hh1, W_ih2, b_ih2, b_hh2,
           W_ihm, b_ihm, b_hhm, W_ihd, b_ihd, b_hhd):
    x = np.asarray(x, np.float32)
    consts = _prepare_consts(
        np.asarray(W_ih1, np.float32), np.asarray(b_ih1, np.float32),
        np.asarray(b_hh1, np.float32), np.asarray(W_ih2, np.float32),
        np.asarray(b_ih2, np.float32), np.asarray(b_hh2, np.float32),
        np.asarray(W_ihm, np.float32), np.asarray(b_ihm, np.float32),
        np.asarray(b_hhm, np.float32), np.asarray(W_ihd, np.float32),
        np.asarray(b_ihd, np.float32), np.asarray(b_hhd, np.float32))
    out, _ = _run(x, consts, trace=False)
    return out


# revision 20
# speedup vs baseline: 1.0336x; 1.0052x over previous
"""DynamicsNet Trainium2 kernel: 4 zero-state LSTM cells, data-parallel on 8 cores.

Reference math per row x[16]:
    h1 = relu(lstm1(x));  h2 = selu(lstm2(h1));  m = tanh(lstmM(h2));
    d = tanh(lstmD(h2));  out = concat([m, d], axis=0)
(zero-state LSTM cell: h = sigmoid(o) * tanh(sigmoid(i) * tanh(g)), f unused)

Layout v2: per core, 10 chunk-streams of 12 lanes each -> partitions 0..119
(vs v1's 8 streams x 16 with gap lanes). Each gate bank is a block-diagonal
matmul; cell1 (16 input feats, 10*16 > 128) splits into two 5-chunk halves
whose stationaries write PSUM partitions [0:60) and [60:120) of the same bank.
Biases ride constant-one partitions: rows 80/81 of the x tiles carry literal
1.0 from HBM; lanes 120/121 of the on-chip H tiles are memset to 1.0 on the
(otherwise idle) gpsimd engine. Bias rows are hi/lo fp16 split -> fp32-grade.

Precision: single-term fp16 matmuls (W16 * x16), fp16 activation outputs and
fp16 elementwise; measured end-to-end rel err ~2.5e-3 vs the 2e-2 gate.
Sigmoid goes through tanh (sigma(z) = (1+tanh(z/2))/2 with the 1/2 folded
into weights); selu refactored to (lambda/2 W)*(max(v,0) + 2a*exp(min(v,0)/2)
- 2a) whose -2a centering exactly cancels the selu alpha-offset in the bias.
Outputs written fp16 (tanh range), upcast on host: halves the output DMA.
"""

from contextlib import ExitStack

import numpy as np

LAMBDA = 1.0507009873554805
ALPHA = 1.6732632423543772
TWOA = float(2.0 * ALPHA)

B, IN, H = 1048576, 16, 12
NCORES = 8
R = B // NCORES          # rows per core
NCHUNK = 10              # chunk streams per core
F = 512                  # free-dim tile
NIT = 26                 # iterations; 10*26*512 = 133120 >= 131072 (pad 2048)
CLEN = NIT * F           # 13312 columns per stream
RPAD = NCHUNK * CLEN     # padded rows per core

_CACHED = {}


def _f16(a):
    return np.asarray(a, np.float64).astype(np.float16)


def _prepare_consts(W_ih1, b_ih1, b_hh1, W_ih2, b_ih2, b_hh2,
                    W_ihm, b_ihm, b_hhm, W_ihd, b_ihd, b_hhd):
    i_s, g_s, o_s = slice(0, 12), slice(24, 36), slice(36, 48)
    L2 = LAMBDA / 2.0

    def _c16(v):
        return np.float16(v).astype(np.float64)

    def _chain_v(aI, aG, aO):
        """fp16-simulated H-lane value from gate-bank consts (relu branch)."""
        ti = _c16(np.tanh(aI))
        tg = _c16(np.tanh(aG))
        to = _c16(np.tanh(aO))
        c2 = _c16((ti + 1.0) * tg)
        tc = _c16(np.tanh(0.5 * c2))
        return float(_c16((to + 1.0) * tc))

    A_I, A_G, A_O = 2.0, 0.6505, 2.0
    v1 = _chain_v(A_I, A_G, A_O)          # H1 const-lane value (lanes 60/61)
    # cell2 const seeds ride H1 lanes (value v1); effective bank consts:
    seed = {0: float(_c16(A_I / v1)), 1: float(_c16(A_G / v1)),
            2: float(_c16(A_O / v1))}
    a2_eff = {b: v1 * seed[b] for b in range(3)}
    vh2 = _chain_v(a2_eff[0], a2_eff[1], a2_eff[2])
    e2c = _c16(2.0 * ALPHA)
    v2 = float(_c16(vh2 + float(e2c) - 2.0 * ALPHA))  # H2 lanes 120/121

    b1 = (b_ih1 + b_hh1).astype(np.float64)
    b2 = (b_ih2 + b_hh2).astype(np.float64)
    bm = (b_ihm + b_hhm).astype(np.float64)
    bd = (b_ihd + b_hhd).astype(np.float64)

    # slot layout in w_dram: [c1A x3 banks, c1B x3, cell2 x3, m x3, d x3]
    w_np = np.zeros((128, 15 * 128), np.float16)

    def put_bias(slot, cols12, bb, scale_lane, r0=120):
        bbv = np.asarray(bb, np.float64) / scale_lane
        hi = _f16(bbv)
        lo = _f16(bbv - hi.astype(np.float64))
        for j0 in cols12:
            w_np[r0, 128 * slot + j0:128 * slot + j0 + 12] = hi
            w_np[r0 + 1, 128 * slot + j0:128 * slot + j0 + 12] = lo

    for b, (gsl, sc) in enumerate(((i_s, 0.5), (g_s, 1.0), (o_s, 0.5))):
        W1b = _f16(W_ih1.astype(np.float64)[gsl] * sc)          # [12, 16]
        bhi = _f16(b1[gsl] * sc)
        blo = _f16(b1[gsl] * sc - bhi.astype(np.float64))
        for half, slot in ((0, b), (1, 3 + b)):
            for j in range(5):
                c0 = 128 * slot + 12 * j
                w_np[16 * j:16 * j + 16, c0:c0 + 12] = W1b.T
                w_np[80, c0:c0 + 12] = bhi
                w_np[81, c0:c0 + 12] = blo
        # A-half const cols 60/61 -> bank lanes 60/61 (seed the H1 const chain)
        w_np[80, 128 * b + 60:128 * b + 62] = np.float16((A_I, A_G, A_O)[b])
        # cell2: input H1 is holey (chunks 0-4 at rows 0..59, hole 60..63
        # with memset-1.0 bias lanes at 60/61, chunks 5-9 at rows 64..123)
        slot = 6 + b
        Wb = _f16(W_ih2.astype(np.float64)[gsl] * sc * 0.5)     # [12, 12]
        b2hi = _f16(b2[gsl] * sc / v1)
        b2lo = _f16(b2[gsl] * sc / v1 - b2hi.astype(np.float64))
        for j in range(NCHUNK):
            r0 = 12 * j if j < 5 else 64 + 12 * (j - 5)
            c0 = 128 * slot + 12 * j
            w_np[r0:r0 + 12, c0:c0 + 12] = Wb.T
            w_np[60, c0:c0 + 12] = b2hi
            w_np[61, c0:c0 + 12] = b2lo
        # cell2 const cols 120/121 (ride H1 lane value v1 -> bank consts)
        w_np[60, 128 * slot + 120:128 * slot + 122] = np.float16(seed[b])
        # m/d: input H2 dense, bias on const lanes 120/121
        for cell, (W, bb) in enumerate(((W_ihm, bm), (W_ihd, bd))):
            slot = 9 + 3 * cell + b
            Wb = _f16(W.astype(np.float64)[gsl] * sc * L2)      # [12, 12]
            for j in range(NCHUNK):
                c0 = 128 * slot + 12 * j
                w_np[12 * j:12 * j + 12, c0:c0 + 12] = Wb.T
        for cell, bb in enumerate((bm, bd)):
            put_bias(9 + 3 * cell + b, range(0, 120, 12), bb[gsl] * sc, v2)
    return (w_np,)


def _build_bass():
    import concourse.bass as bass
    import concourse.mybir as mybir
    import concourse.tile as tile

    fp32 = mybir.dt.float32
    fp16 = mybir.dt.float16
    Tanh = mybir.ActivationFunctionType.Tanh
    Exp = mybir.ActivationFunctionType.Exp
    ADD = mybir.AluOpType.add
    MULT = mybir.AluOpType.mult
    MAX = mybir.AluOpType.max
    MIN = mybir.AluOpType.min

    P = 120              # dense data partitions (cell2/m/d outputs)
    PA = 124             # holey cell1 span: [0:60) + hole + [64:124)

    nc = bass.Bass(name="dynet2")
    x_dev = nc.dram_tensor("x_dev", [82, 2 * CLEN], fp16,
                           kind="ExternalInput")
    w_dram = nc.dram_tensor("w_dram", [128, 15 * 128], fp16,
                            kind="ExternalInput")
    md_dev = nc.dram_tensor("md_dev", [P, 2 * CLEN], fp16,
                            kind="ExternalOutput")

    with tile.TileContext(nc) as tc, ExitStack() as ctx:
        const_p = ctx.enter_context(tc.tile_pool(name="const", bufs=1))
        xp = ctx.enter_context(tc.tile_pool(name="x", bufs=3))
        Tp = ctx.enter_context(tc.tile_pool(name="T", bufs=3))
        smallp = ctx.enter_context(tc.tile_pool(name="small", bufs=4))
        hp = ctx.enter_context(tc.tile_pool(name="h", bufs=4))
        psp = ctx.enter_context(tc.tile_pool(name="ps", bufs=1, space="PSUM"))

        wsb = const_p.tile([128, 15 * 128], fp16)
        nc.sync.dma_start(wsb[:], w_dram[:])
        ebt = const_p.tile([128, 1], fp32)
        nc.vector.memset(ebt[:], float(np.log(2.0 * ALPHA)))

        # PSUM ring of 8 banks; each stage claims 3 consecutive (mod 8).
        # cell1's A-half spans 64 cols (62/63 are zero weight columns), so
        # every lane a matmul ever reads back is written finite first.
        pp_all = psp.tile([128, 4096], fp32)

        def bank(s, b):
            w = ((3 * s + b) % 8) * 512
            return pp_all[:, w:w + 512]

        def mms_h(s, slot0, rhs, nrow, ncol=120):
            """3 block-diag matmuls (one per bank) for an h-input cell."""
            for b in range(3):
                w = 128 * (slot0 + b)
                nc.tensor.matmul(
                    bank(s, b)[0:ncol, :], wsb[0:nrow, w:w + ncol],
                    rhs[0:nrow, :], start=True, stop=True)

        def mms_x(s, xt):
            """cell1: two 5-chunk halves per bank -> partitions [0:62),[64:124)."""
            for b in range(3):
                wa, wb = 128 * b, 128 * (3 + b)
                nc.tensor.matmul(
                    bank(s, b)[0:64, :], wsb[0:82, wa:wa + 64],
                    xt[0:82, 0:F], start=True, stop=True)
                nc.tensor.matmul(
                    bank(s, b)[64:124, :], wsb[0:82, wb:wb + 60],
                    xt[0:82, F:2 * F], start=True, stop=True)

        def ring_act(T, tcol0, base, nq, pm=P):
            """tanh over ring slots [base, base+nq) -> T[:, tcol0:+512*nq]."""
            done = 0
            while done < nq:
                b0 = (base + done) % 8
                n1 = min(nq - done, 8 - b0)
                nc.scalar.activation(
                    T[0:pm, tcol0 + 512 * done:tcol0 + 512 * (done + n1)],
                    pp_all[0:pm, 512 * b0:512 * (b0 + n1)], Tanh)
                done += n1

        def gate_act(s, T, pm=P):
            ring_act(T, 0, (3 * s) % 8, 3, pm=pm)

        xa = {}
        H1 = {}
        H2 = {}
        sctr = 0
        for k in range(NIT + 3):
            if k < NIT:
                xa[k] = xp.tile([82, 2 * F], fp16, tag="xa", name=f"xa{k}")
                nc.sync.dma_start(xa[k][:],
                                  x_dev[:, 2 * F * k:2 * F * (k + 1)])

            # fused tanh in/out: [0:1024]=hmd(k-2), [1024:2048]=c2both(k-1,k)
            fin = smallp.tile([PA, 2048], fp16, tag="fin", name=f"fin{k}")
            fout = smallp.tile([PA, 2048], fp16, tag="fout", name=f"fout{k}")
            hmd = fin[:, 0:1024]
            omd = fout[:, 0:1024]

            # --- stage m/d for iteration k-3 ---
            if 3 <= k:
                it = k - 3
                sm, sd = sctr, sctr + 1
                sctr += 2
                mms_h(sm, 9, H2[it][:], 122)
                mms_h(sd, 12, H2[it][:], 122)
                Tmd = Tp.tile([P, 3072], fp16, tag="Tmd", bufs=2)
                Tm = Tmd[:, 0:1536]
                Td = Tmd[:, 1536:3072]
                c2md = smallp.tile([P, 1024], fp16, tag="c2md")
                tcmd = smallp.tile([P, 1024], fp16, tag="tcmd")
                if (3 * sm) % 8 <= 2:
                    ring_act(Tmd, 0, (3 * sm) % 8, 6)
                else:
                    gate_act(sm, Tm)
                    gate_act(sd, Td)
                nc.vector.scalar_tensor_tensor(
                    c2md[:, 0:512], Tm[:, 0:512], 1.0, Tm[:, 512:1024],
                    op0=ADD, op1=MULT)
                nc.vector.scalar_tensor_tensor(
                    c2md[:, 512:1024], Td[:, 0:512], 1.0, Td[:, 512:1024],
                    op0=ADD, op1=MULT)
                nc.scalar.activation(tcmd[:, :], c2md[:, :], Tanh, scale=0.5)
                nc.vector.scalar_tensor_tensor(
                    fin[0:P, 0:512], Tm[:, 1024:1536], 1.0, tcmd[:, 0:512],
                    op0=ADD, op1=MULT)
                nc.vector.scalar_tensor_tensor(
                    fin[0:P, 512:1024], Td[:, 1024:1536], 1.0,
                    tcmd[:, 512:1024], op0=ADD, op1=MULT)
                del H2[it]
                md_out_it = it

            # --- stage cell2 for iteration k-1 ---
            if 1 <= k <= NIT:
                it = k - 1
                s2 = sctr
                sctr += 1
                mms_h(s2, 6, H1[it][:], 124, ncol=122)
                T2 = Tp.tile([PA, 1536], fp16, tag="T")
                gate_act(s2, T2, pm=122)
                nc.vector.scalar_tensor_tensor(
                    fin[0:122, 1024:1536], T2[0:122, 0:512], 1.0,
                    T2[0:122, 512:1024], op0=ADD, op1=MULT)

            # --- stage cell1 for iteration k ---
            if k < NIT:
                s1 = sctr
                sctr += 1
                mms_x(s1, xa[k])
                T1 = Tp.tile([PA, 1536], fp16, tag="T")
                gate_act(s1, T1, pm=PA)
                nc.vector.scalar_tensor_tensor(
                    fin[0:PA, 1536:2048], T1[0:PA, 0:512], 1.0,
                    T1[0:PA, 512:1024], op0=ADD, op1=MULT)

            # merged tanh(0.5*x) over [hmd(k-3) | c2_2(k-1) | c2_1(k)]
            lo = 0 if 3 <= k else 1024
            hi = 2048 if k < NIT else (1536 if k <= NIT else 1024)
            if k == 0:
                lo = 1536
            nc.scalar.activation(fout[0:PA, lo:hi], fin[0:PA, lo:hi], Tanh,
                                 scale=0.5)
            if 3 <= k:
                it = md_out_it
                nc.gpsimd.dma_start(md_dev[:, 2 * F * it:2 * F * (it + 1)],
                                    omd[0:P, 0:1024])

            # --- cell2 epilogue: selu -> H2 fp16 (+const lanes) ---
            if 1 <= k <= NIT:
                it = k - 1
                h2x2 = smallp.tile([122, F], fp16, tag="h2x2",
                                   name=f"h2x2_{k}")
                nc.vector.scalar_tensor_tensor(
                    h2x2[:, :], T2[0:122, 1024:1536], 1.0,
                    fout[0:122, 1024:1536], op0=ADD, op1=MULT)
                m0t = smallp.tile([122, F], fp16, tag="m0")
                e2t = smallp.tile([122, F], fp16, tag="e2")
                tmp2 = smallp.tile([122, F], fp32, tag="tmp2")
                h2t = hp.tile([128, F], fp16, tag="H2", name=f"H2_{it}")
                nc.vector.tensor_scalar_min(m0t[:, :], h2x2[:, :], 0.0)
                nc.scalar.activation(e2t[:, :], m0t[:, :], Exp,
                                     bias=ebt[0:122, :], scale=0.5)
                nc.vector.scalar_tensor_tensor(
                    tmp2[:, :], h2x2[:, :], 0.0, e2t[:, :], op0=MAX, op1=ADD)
                nc.vector.tensor_scalar_sub(h2t[0:122, :], tmp2[:, :], TWOA)
                H2[it] = h2t
                del H1[it]

            # --- cell1 epilogue: relu -> H1 fp16 (+const lanes) ---
            if k < NIT:
                h2x1 = smallp.tile([PA, F], fp16, tag="h2x1",
                                   name=f"h2x1_{k}")
                nc.vector.scalar_tensor_tensor(
                    h2x1[:, :], T1[0:PA, 1024:1536], 1.0,
                    fout[0:PA, 1536:2048], op0=ADD, op1=MULT)
                h1t = hp.tile([128, F], fp16, tag="H1", name=f"H1_{k}")
                nc.vector.tensor_scalar_max(h1t[0:PA, :], h2x1[:, :], 0.0)
                H1[k] = h1t
                del xa[k]

    _legalize_waits(nc)
    return nc


def _legalize_waits(nc):
    """Split multi-wait instructions into single-wait same-engine NoOps
    (the cayman ISA has one sync-wait slot per instruction)."""
    import concourse.mybir as mybir
    n = 0
    for func in nc.m.functions:
        for blk in func.blocks:
            out = []
            changed = False
            for inst in blk.instructions:
                si = inst.sync_info
                waits = list(si.on_wait) if si is not None and si.on_wait else []
                if len(waits) > 1:
                    changed = True
                    for w in waits[:-1]:
                        n += 1
                        nop = mybir.InstNoOp(name=f"legw-{n}", ins=[], outs=[])
                        nop.engine = inst.engine
                        nop.sync_info = mybir.SyncInfo(on_wait=[w], on_update=[])
                        out.append(nop)
                    inst.sync_info = mybir.SyncInfo(
                        on_wait=[waits[-1]],
                        on_update=list(si.on_update) if si.on_update else [])
                out.append(inst)
            if changed:
                blk.instructions = out
    return n


def _run(x, consts, trace=False):
    from concourse.bass_utils import run_bass_kernel_spmd

    if "nc" not in _CACHED:
        _CACHED["nc"] = _build_bass()
    nc = _CACHED["nc"]
    (w_np,) = consts

    in_maps = []
    for c in range(NCORES):
        xpad = np.zeros((RPAD, IN), np.float32)
        xpad[:R] = x[c * R:(c + 1) * R]
        blk = np.ascontiguousarray(
            xpad.reshape(NCHUNK, CLEN, IN).transpose(0, 2, 1))  # [10,16,CLEN]
        xa = np.empty((82, CLEN), np.float16)
        xb = np.empty((82, CLEN), np.float16)
        for j in range(5):
            xa[16 * j:16 * j + 16] = blk[j]
            xb[16 * j:16 * j + 16] = blk[5 + j]
        xa[80:82] = 1.0
        xb[80:82] = 1.0
        xdv = np.empty((82, 2 * CLEN), np.float16)
        xv = xdv.reshape(82, NIT, 2 * F)
        xv[:, :, 0:F] = xa.reshape(82, NIT, F)
        xv[:, :, F:2 * F] = xb.reshape(82, NIT, F)
        in_maps.append({"x_dev": xdv, "w_dram": w_np})

    res = run_bass_kernel_spmd(nc, in_maps, core_ids=list(range(NCORES)),
                               trace=trace)

    out = np.empty((2 * B, H), np.float32)
    for c in range(NCORES):
        md = res.results[c]["md_dev"].reshape(120, NIT, 2 * F)
        for half, base in ((0, 0), (1, B)):
            dev = np.ascontiguousarray(
                md[:, :, half * F:(half + 1) * F]).reshape(120, CLEN)
            full = dev.reshape(NCHUNK, H, CLEN).transpose(0, 2, 1)
            out[base + c * R: base + (c + 1) * R] = (
                full.reshape(RPAD, H)[:R].astype(np.float32))
    return out, res


def kernel(x, W_ih1, b_ih1, b_hh1, W_ih2, b_ih2, b_hh2,
           W_ihm, b_ihm, b_hhm, W_ihd, b_ihd, b_hhd):
    x = np.asarray(x, np.float32)
    consts = _prepare_consts(
        np.asarray(W_ih1, np.float32), np.asarray(b_ih1, np.float32),
        np.asarray(b_hh1, np.float32), np.asarray(W_ih2, np.float32),
        np.asarray(b_ih2, np.float32), np.asarray(b_hh2, np.float32),
        np.asarray(W_ihm, np.float32), np.asarray(b_ihm, np.float32),
        np.asarray(b_hhm, np.float32), np.asarray(W_ihd, np.float32),
        np.asarray(b_ihd, np.float32), np.asarray(b_hhd, np.float32))
    out, _ = _run(x, consts, trace=False)
    return out
